# revision 6
# baseline (speedup 1.0000x reference)
"""MeshConv-transpose Trainium2 kernel, v3.

out[b,:,n] = (identity @ c0 + L_spmm @ c1 + EW_spmm @ c2 + NS_spmm @ c3 + bias)^T

Strategy (8 NeuronCores): each core holds ALL 8 batches and 1/8 of the dests.
- Phase 1: channel transform on PE: tables T0 = [x;1] @ c0 and T123 = [x;1] @
  c(1..3), rows packed [vertex, 8 batches x 64 ch] bf16 (1KB) in HBM scratch.
- Phase 2 per dest tile (128 dests on partitions, degree-sorted + dealt to
  8 shards): gpsimd.dma_gather pulls 1KB rows (slot 0 = identity from T0,
  cols >= NVPREV fold into a per-dest pad-sum handled by a tiny [5,128]@
  [5,512] PE matmul), then a fused DVE multiply-accumulate chain
  (scalar_tensor_tensor) reduces slots in fp32 — no tree fold, one pass.
- 1KB descriptors halve the Q7 descriptor-generation work vs 4-batch rows;
  host un-permutes the bf16 output.
"""
import numpy as np

import concourse.bass as bass
import concourse.mybir as mybir
import concourse.tile as tile
from concourse import library_config
from concourse.bass_utils import run_bass_kernel_spmd
from concourse.library_overlay import lower_extended_insts

# ---- problem constants (hardcoded per harness contract) ----
NV = 40962
NVPREV = 10242
B = 8
C = 64

NSH = 8            # dest shards = cores
NVQ = 10368        # table rows per op (81*128 >= NVPREV+1)
DPC = 5248         # dests per core (41*128)
NPAD = NSH * DPC   # padded dest count 41984
NT = DPC // 128    # 41 dest tiles
EW = B * C         # elem width per table row (512 bf16 = 1KB)

f32 = mybir.dt.float32
bf16 = mybir.dt.bfloat16
i16 = mybir.dt.int16
NP_BF16 = mybir.dt.np(bf16)


def _fix_multiwait(nc, max_waits=1):
    """This walrus build accepts one sem-wait per instruction; hoist extras
    onto same-engine no-ops spliced before the instruction."""
    for f in nc.m.functions:
        for bb in f.blocks:
            out, changed = [], False
            for inst in bb.instructions:
                si = inst.sync_info
                waits = list(si.on_wait) if si and si.on_wait else []
                if len(waits) > max_waits:
                    for w in waits[:-max_waits]:
                        nop = mybir.InstNoOp(
                            name=nc.get_next_instruction_name(),
                            engine=inst.engine, ins=[], outs=[],
                            sync_info=mybir.SyncInfo(on_wait=[w], on_update=[]),
                        )
                        nc.register_instruction(nop)
                        out.append(nop)
                    si.on_wait = waits[-max_waits:]
                    changed = True
                out.append(inst)
            if changed:
                bb.instructions = out


def _wrap_idx(idx_flat):
    """Pack a flat index list into the dma_gather idx tile layout:
    wrapped into 16 partitions, replicated to 8 Q7 cores."""
    n = len(idx_flat)
    w = np.zeros((16, n // 16), np.int16)
    q = np.arange(n)
    w[q % 16, q // 16] = idx_flat
    return np.tile(w, (8, 1))  # [128, n//16]


def _preprocess(x, L_cols, L_vals, EW_cols, EW_vals, NS_cols, NS_vals, coeffs, bias):
    cols_ops = [np.asarray(L_cols), np.asarray(EW_cols), np.asarray(NS_cols)]
    vals_ops = [np.asarray(L_vals, np.float32), np.asarray(EW_vals, np.float32),
                np.asarray(NS_vals, np.float32)]

    real_masks = [c < NVPREV for c in cols_ops]
    deg_ops = [m.sum(1) for m in real_masks]
    deg = sum(deg_ops)
    s_pad = [np.where(~m, v, 0).sum(1).astype(np.float32)
             for m, v in zip(real_masks, vals_ops)]

    # ELL pack of real edges per dest, ops concatenated (t123 row k*NVQ+col)
    dmax = int(deg.max())
    eidx = np.zeros((NV, dmax), np.int16)
    evals = np.zeros((NV, dmax), np.float32)
    pos = np.zeros(NV, np.int64)
    for k in range(3):
        m = real_masks[k]
        r = m.cumsum(1) - 1 + pos[:, None]
        rows, _ = np.nonzero(m)
        eidx[rows, r[m]] = (cols_ops[k][m] + k * NVQ).astype(np.int16)
        evals[rows, r[m]] = vals_ops[k][m]
        pos += deg_ops[k]

    deg_p = np.concatenate([deg, np.full(NPAD - NV, -1)])
    order = np.argsort(-deg_p, kind="stable")
    pis = [order[c::NSH] for c in range(NSH)]

    S_t = np.zeros(NT, np.int64)
    for c in range(NSH):
        d = np.clip(deg_p[pis[c]], 0, None).reshape(NT, 128)
        S_t = np.maximum(S_t, 1 + d.max(1))

    shards = []
    for c in range(NSH):
        pi = pis[c]
        idx0 = np.zeros((NT, 128, 8), np.int16)
        idx123_cols, vals_cols = [], []
        s5 = np.zeros((NT, 5, 128), np.float32)
        for t in range(NT):
            p_ids = pi[t * 128:(t + 1) * 128]
            st = int(S_t[t])
            safe = np.minimum(p_ids, NV - 1)
            real = p_ids < NV
            id_idx = np.where(p_ids < NVPREV, p_ids, NVPREV).astype(np.int16)
            idx0[t] = _wrap_idx(np.where(real, id_idx, 0))
            bi = eidx[safe][:, :st - 1] * real[:, None]      # [128, st-1]
            bv = evals[safe][:, :st - 1] * real[:, None]
            idx123_cols.append(_wrap_idx(bi.T.ravel()))
            vt = np.zeros((128, st), np.float32)
            vt[:, 0] = real
            vt[:, 1:] = bv
            vals_cols.append(vt)
            for r in range(3):
                s5[t, r + 1] = np.where(real, s_pad[r][safe], 0)
            s5[t, 4] = real
        shards.append(dict(
            pi=pi,
            idx0=idx0,
            idx123=np.concatenate(idx123_cols, axis=1),
            vals=np.concatenate(vals_cols, axis=1),
            s5=s5,
        ))

    coeffs = np.asarray(coeffs, np.float32)
    callw = np.concatenate([coeffs[k] for k in range(4)], axis=1).astype(NP_BF16)
    csum = coeffs.sum(axis=1)
    cs = np.zeros((5, EW), np.float32)
    for k in range(1, 4):
        cs[k] = np.tile(csum[k], B)
    cs[4] = np.tile(np.asarray(bias, np.float32), B)

    x = np.asarray(x, np.float32)
    xq = np.zeros((B, C, NVQ), np.float32)
    xq[:, :, :NVPREV] = x
    xq[:, :, NVPREV] = 1.0
    xq = xq.astype(NP_BF16)

    return shards, xq, callw, cs, S_t


def _build_program(S_t, wtot, stot, phase1=True, phase2=True, hw_loop=0,
                   n_queues=1):
    nc = bass.Bass(num_swdge_queues=n_queues)
    xq_ext = nc.declare_dram_parameter("xq", [B, C, NVQ], bf16, isOutput=False)
    callw_ext = nc.declare_dram_parameter("callw", [C, 4 * C], bf16, isOutput=False)
    cs_ext = nc.declare_dram_parameter("cs", [5, EW], f32, isOutput=False)
    idx0_ext = nc.declare_dram_parameter("idx0", [NT, 128, 8], i16, isOutput=False)
    idx123_ext = nc.declare_dram_parameter("idx123", [128, wtot], i16, isOutput=False)
    vals_ext = nc.declare_dram_parameter("vals", [128, stot], f32, isOutput=False)
    s5_ext = nc.declare_dram_parameter("s5", [NT, 5, 128], f32, isOutput=False)
    out_ext = nc.declare_dram_parameter("out", [DPC, EW], bf16, isOutput=True)

    t0_dram = nc.dram_tensor("t0_scratch", [NVQ, EW], bf16)
    t123_dram = nc.dram_tensor("t123_scratch", [3 * NVQ, EW], bf16)

    s_max = int(S_t.max())

    with tile.TileContext(nc) as tc:
        with (
            tc.tile_pool(name="const", bufs=1) as constp,
            tc.tile_pool(name="psum", bufs=4, space="PSUM") as psum,
        ):
            nc.gpsimd.load_library(library_config.mlp)
            callw_t = constp.tile([C, 4 * C], bf16)
            cs_t = constp.tile([5, EW], f32)
            nc.sync.dma_start(callw_t[:], callw_ext[:])
            nc.sync.dma_start(cs_t[:], cs_ext[:])

            gq = [0]

            def _phase1():
                with (
                    tc.tile_pool(name="xqp", bufs=3) as xqp,
                    tc.tile_pool(name="zstage", bufs=3) as zst,
                ):
                    t123_v = t123_dram[:].rearrange("(k v) e -> k v e", k=3)
                    for vt in range(NVQ // 128):
                        sl = slice(vt * 128, (vt + 1) * 128)
                        st0 = zst.tile([128, EW], bf16, tag="st0")
                        st123 = zst.tile([128, 3, EW], bf16, tag="st123")
                        xt = xqp.tile([C, B, 128], bf16, tag="xq")
                        nc.sync.dma_start(xt[:],
                                          xq_ext[:, :, sl].transpose([1, 0, 2]))
                        for bb in range(B):
                            ps = psum.tile([128, 4 * C], f32, tag="zps")
                            nc.tensor.matmul(ps[:], xt[:, bb, :], callw_t[:],
                                             start=True, stop=True)
                            ceng = (nc.scalar.copy if bb % 2 == 0
                                    else nc.vector.tensor_copy)
                            ceng(st0[:, bb * C:(bb + 1) * C], ps[:, 0:C])
                            ceng(
                                st123[:, :, bb * C:(bb + 1) * C],
                                ps[:, C:4 * C].rearrange("p (k c) -> p k c", k=3),
                            )
                        nc.sync.dma_start(t0_dram[sl], st0[:])
                        nc.sync.dma_start(
                            t123_v[:, sl, :].transpose([1, 0, 2]), st123[:])

            reg_cache = {}

            def nreg(v):
                if v not in reg_cache:
                    reg_cache[v] = nc.gpsimd.to_reg(v)
                return reg_cache[v]

            def _gather(out_ap, tab, idxs, n):
                q = gq[0] % n_queues
                gq[0] += 1
                nc.gpsimd.dma_gather(out_ap, tab, idxs, num_idxs=n,
                                     num_idxs_reg=nreg(n), elem_size=EW,
                                     queue_num=q, single_packet=True)

            def _phase2():
                do_gather = phase2 in (True, "gather")
                do_compute = phase2 in (True, "compute")
                GRP = 8
                with (
                    tc.tile_pool(name="work", bufs=2) as work,
                    tc.tile_pool(name="accp", bufs=2) as accp,
                    tc.tile_pool(name="psc", bufs=2, space="PSUM") as pscp,
                ):
                    woff = 0
                    voff = 0
                    gw = gv = 0
                    for t in range(NT):
                        st = int(S_t[t])
                        wt = (st - 1) * 8
                        if t % GRP == 0:
                            tn = min(GRP, NT - t)
                            gwid = sum((int(S_t[u]) - 1) * 8
                                       for u in range(t, t + tn))
                            gsl = sum(int(S_t[u]) for u in range(t, t + tn))
                            idx0_g = work.tile([128, GRP, 8], i16, tag="idx0")
                            idx123_g = work.tile([128, max(gwid, 1)], i16,
                                                 tag="idx123")
                            vals_g = work.tile([128, gsl], f32, tag="vals")
                            s5_g = work.tile([5, GRP, 128], f32, tag="s5")
                            nc.sync.dma_start(
                                idx0_g[:, :tn, :],
                                idx0_ext[t:t + tn].transpose([1, 0, 2]))
                            if gwid:
                                nc.sync.dma_start(idx123_g[:, :gwid],
                                                  idx123_ext[:, woff:woff + gwid])
                            nc.sync.dma_start(vals_g[:, :gsl],
                                              vals_ext[:, voff:voff + gsl])
                            nc.sync.dma_start(
                                s5_g[:, :tn, :],
                                s5_ext[t:t + tn].transpose([1, 0, 2]))
                            gw = gv = 0
                        ti = t % GRP
                        idx0_t = idx0_g[:, ti, :]
                        s5_t = s5_g[:, ti, :]

                        G = work.tile([128, s_max, EW], bf16, tag="G")
                        if do_gather:
                            _gather(G[:, 0:1, :], t0_dram[:], idx0_t, 128)
                            d0 = 1
                            while d0 < st:
                                dn = min(st - d0, 8)
                                c0 = (d0 - 1) * 8
                                _gather(G[:, d0:d0 + dn, :], t123_dram[:],
                                        idx123_g[:, gw + c0:gw + c0 + dn * 8],
                                        dn * 128)
                                d0 += dn
                        if not do_compute:
                            nc.gpsimd.dma_start(out_ext[t * 128:(t + 1) * 128],
                                                G[:, 0, :])
                            woff += wt
                            voff += st
                            continue
                        # two interleaved fp32 MAC chains (even/odd slots):
                        # fused scale+accumulate, half dependency depth
                        acc = [[accp.tile([128, EW], f32, tag=f"acc{h}{i}",
                                          name=f"acc{h}{i}")
                                for i in range(2)] for h in range(2)]
                        cur = [None, None]
                        for h in range(2):
                            if h >= st:
                                break
                            nc.vector.tensor_scalar_mul(
                                acc[h][0][:], G[:, h, :],
                                vals_g[:, gv + h:gv + h + 1])
                            cur[h] = 0
                        for s in range(2, st):
                            h = s % 2
                            a, b = acc[h][cur[h]], acc[h][1 - cur[h]]
                            nc.vector.scalar_tensor_tensor(
                                out=b[:], in0=G[:, s, :],
                                scalar=vals_g[:, gv + s:gv + s + 1],
                                in1=a[:],
                                op0=mybir.AluOpType.mult,
                                op1=mybir.AluOpType.add)
                            cur[h] = 1 - cur[h]
                        ps2 = pscp.tile([128, EW], f32, tag="cps")
                        nc.tensor.matmul(ps2[:], s5_t, cs_t[:], start=True,
                                         stop=True)
                        outt = work.tile([128, EW], bf16, tag="outt")
                        if cur[1] is not None:
                            facc = accp.tile([128, EW], f32, tag="facc")
                            nc.vector.tensor_tensor(
                                out=facc[:], in0=acc[0][cur[0]][:],
                                in1=acc[1][cur[1]][:], op=mybir.AluOpType.add)
                            fin = facc
                        else:
                            fin = acc[0][cur[0]]
                        nc.vector.tensor_tensor(out=outt[:], in0=fin[:],
                                                in1=ps2[:],
                                                op=mybir.AluOpType.add)
                        nc.sync.dma_start(out_ext[t * 128:(t + 1) * 128],
                                          outt[:])
                        woff += wt
                        voff += st
                        gw += wt
                        gv += st

            def _body():
                if phase1:
                    _phase1()
                if phase2:
                    _phase2()

            if hw_loop:
                with tc.For_i(0, hw_loop):
                    _body()
            else:
                _body()

    lower_extended_insts(nc)
    _fix_multiwait(nc)
    return nc


def kernel(x, L_cols, L_vals, EW_cols, EW_vals, NS_cols, NS_vals, coeffs, bias):
    shards, xq, callw, cs, S_t = _preprocess(
        x, L_cols, L_vals, EW_cols, EW_vals, NS_cols, NS_vals, coeffs, bias)

    wtot = shards[0]["idx123"].shape[1]
    stot = shards[0]["vals"].shape[1]
    assert all(sd["idx123"].shape[1] == wtot for sd in shards)

    nc = _build_program(S_t, wtot, stot, n_queues=2)

    in_maps = []
    for c in range(NSH):
        sd = shards[c]
        in_maps.append({
            "xq": xq,
            "callw": callw,
            "cs": cs,
            "idx0": sd["idx0"],
            "idx123": sd["idx123"],
            "vals": sd["vals"],
            "s5": sd["s5"],
        })

    res = run_bass_kernel_spmd(nc, in_maps, list(range(NSH)))

    out = np.zeros((B, C, NV), np.float32)
    for c in range(NSH):
        pi = shards[c]["pi"]
        valid = pi < NV
        o = np.asarray(res.results[c]["out"]).astype(np.float32)
        rows = o[valid].reshape(-1, B, C)      # [nvalid, b, ch]
        out[:, :, pi[valid]] = rows.transpose(1, 2, 0)
    return out
